# revision 17
# baseline (speedup 1.0000x reference)
"""Trainium2 Bass kernel for the CustomLSTMCell problem.

B=64, T=1024, D=H=512.  Data-parallel over batch: 8 NeuronCores x 8 rows.

Per-core plan (matmul operands bf16 (or fp8 weights), state fp32):
  Host pre-transposes weights/x so no on-chip transposes are needed.
  Gate-chunk order is chosen so the per-step gate PSUM splits into two
  independently-completing half blocks:
    block A: h-dims 0..255   chunks [f0,f1,i0,i1,o0,o1,2g0,2g1]
    block B: h-dims 256..511 chunks [f2,f3,i2,i3,o2,o3,2g2,2g3]
  The g~ gate weights/biases are pre-scaled by 2 so ONE Sigmoid ACT per
  block covers all four gates: tanh(x) = 2*sigmoid(2x) - 1.

  Phase 1: x_proj[g,p,(t,b)] = Wx.T @ x + b  -> bf16 DRAM scratch.
  Phase 2: 1024 sequential steps.  Per step:
    - 2 identity matmuls deposit x_proj_t into psA/psB (start=True)
    - 32+32 (LDWEIGHTS+MATMUL) pairs accumulate Wh @ h_{t-1}; within a
      block the contraction chunks k go oldest-first so the next step's
      stream needs fresh h-chunks as late as possible; block A stops at
      half-stream so its activation chain overlaps block B's stream.
    - per block: Sigmoid ACT -> DVE (2s-1, muls, add) -> Tanh ACT -> DVE
      updates c (fp32) and h (bf16), software-pipelined across steps.
  x_proj is staged from DRAM in 128-step chunks, double buffered.
"""

import numpy as np
import ml_dtypes

import concourse.bass as bass
import concourse.bacc as bacc
import concourse.mybir as mybir
import concourse.tile as tile
import concourse.bass_utils as bass_utils

BF16 = mybir.dt.bfloat16
FP8 = mybir.dt.float8e4
F32 = mybir.dt.float32
AF = mybir.ActivationFunctionType
OP = mybir.AluOpType
npbf16 = ml_dtypes.bfloat16
npfp8 = ml_dtypes.float8_e4m3

B, T, D, H = 64, 1024, 512, 512
NC = 8
BPC = B // NC            # 8 batch rows per core
G = 4 * H                # 2048 gate rows
KC = D // 128            # 4 contraction chunks
GC = G // 128            # 16 gate chunks
CHUNK = 128              # timesteps per x_proj staging chunk
TG = 512                 # (t,b) columns per phase-1 psum tile

# gate-chunk permutation: orig chunk id = gate*4 + k, gates [f,i,o,g]
PERM = [0, 1, 4, 5, 8, 9, 12, 13, 2, 3, 6, 7, 10, 11, 14, 15]

_CACHE = {}


def _build(t_steps, wdt, chunk=CHUNK):
    nc = bacc.Bacc(
        "TRN2",
        target_bir_lowering=False,
        debug=False,
        enable_asserts=False,
        num_devices=NC,
    )
    nchunk = t_steps // chunk
    tg = min(TG, t_steps * BPC)
    ntg = (t_steps * BPC) // tg

    xT_d = nc.dram_tensor("xT", [KC, 128, t_steps * BPC], BF16, kind="ExternalInput")
    whT_d = nc.dram_tensor("whT", [KC, 128, G], wdt, kind="ExternalInput")
    wxT_d = nc.dram_tensor("wxT", [KC, 128, G], BF16, kind="ExternalInput")
    bias_d = nc.dram_tensor("bias", [128, GC], F32, kind="ExternalInput")
    ident_d = nc.dram_tensor("ident", [128, 128], BF16, kind="ExternalInput")
    hout_d = nc.dram_tensor("hout", [128, KC * BPC], F32, kind="ExternalOutput")

    with tile.TileContext(nc) as tc:
        with (
            tc.tile_pool(name="wpool", bufs=1) as wpool,
            tc.tile_pool(name="xpool", bufs=1) as xpool,
            tc.tile_pool(name="xpd", bufs=1, space="DRAM") as dpool,
            tc.tile_pool(name="p1ps", bufs=2, space="PSUM") as p1ps,
            tc.tile_pool(name="p1out", bufs=4) as p1out,
            tc.tile_pool(name="stage", bufs=2) as spool,
            tc.tile_pool(name="gps", bufs=2, space="PSUM") as gps,
            tc.tile_pool(name="state", bufs=1) as st,
        ):
            # ---- resident tensors ----
            whT = wpool.tile([128, KC * G], wdt)
            wxT = wpool.tile([128, KC * G], BF16)
            biasr = wpool.tile([128, GC], F32)
            ident = wpool.tile([128, 128], BF16)
            for k in range(KC):
                nc.sync.dma_start(whT[:, k * G:(k + 1) * G], whT_d[k])
                nc.sync.dma_start(wxT[:, k * G:(k + 1) * G], wxT_d[k])
            nc.sync.dma_start(biasr[:], bias_d[:])
            nc.sync.dma_start(ident[:], ident_d[:])

            xT = xpool.tile([128, KC * t_steps * BPC], BF16)
            W = t_steps * BPC
            for k in range(KC):
                nc.sync.dma_start(xT[:, k * W:(k + 1) * W], xT_d[k])

            xp_d = dpool.tile([GC, 128, t_steps * BPC], BF16)

            # ---- phase 1: x projection (micro-op emitters) ----
            # A "group" is one (tgi, g) output tile: 4 accumulating MMs,
            # 4 bias sub-adds of 128 cols, 1 DMA out.  The first two
            # chunks' groups run upfront; later chunks' groups are
            # interleaved into the recurrence steps (PE/DVE idle time).
            p1_state = {}

            def p1_mm(tgi, g, k):
                if k == 0:
                    p1_state[(tgi, g)] = (
                        p1ps.tile([128, tg], F32, name="p1ps_t"),
                        p1out.tile([128, tg], BF16, name="p1ev_t"),
                    )
                ps, _ = p1_state[(tgi, g)]
                nc.tensor.matmul(
                    ps[:],
                    wxT[:, k * G + g * 128: k * G + (g + 1) * 128],
                    xT[:, k * W + tgi * tg: k * W + (tgi + 1) * tg],
                    start=(k == 0),
                    stop=(k == KC - 1),
                )

            def p1_bias(tgi, g, j):
                # 64-col sub-ops bound how long a straddling phase-1 op
                # can delay the recurrence chain on the vector engine
                ps, ev = p1_state[(tgi, g)]
                q = tg // 8
                nc.vector.tensor_scalar_add(
                    ev[:, j * q:(j + 1) * q], ps[:, j * q:(j + 1) * q],
                    biasr[:, g:g + 1])

            def p1_dma(tgi, g):
                _, ev = p1_state.pop((tgi, g))
                nc.sync.dma_start(xp_d[g, :, tgi * tg:(tgi + 1) * tg], ev[:])

            def p1_group(tgi, g):
                for k in range(KC):
                    p1_mm(tgi, g, k)
                for j in range(8):
                    p1_bias(tgi, g, j)
                p1_dma(tgi, g)

            tgi_per_chunk = (chunk * BPC) // tg
            up_chunks = min(2, nchunk)
            for tgi in range(up_chunks * tgi_per_chunk):
                for g in range(GC):
                    p1_group(tgi, g)

            def p1_micro_ops(n):
                """Micro-op list for source chunk n, consumed one per step."""
                ops = []
                base = n * tgi_per_chunk
                ngroups = tgi_per_chunk * GC
                for gi in range(ngroups):
                    tgi, g = base + gi // GC, gi % GC
                    for k in range(KC):
                        ops.append((4 * gi + k, lambda t=tgi, gg=g, kk=k:
                                    p1_mm(t, gg, kk)))
                    for j in range(8):
                        ops.append((4 * gi + 5 + j // 2,
                                    lambda t=tgi, gg=g, jj=j:
                                    p1_bias(t, gg, jj)))
                    ops.append((4 * gi + 9, lambda t=tgi, gg=g: p1_dma(t, gg)))
                ops.sort(key=lambda x: x[0])
                return ops

            # ---- phase 2: recurrence ----
            HB = 2 * BPC  # 16: free width of one h half (2 k-chunks x 8 b)
            # per-block state, double-buffered by step parity where written
            # by one engine and read by another across steps
            sig_v = [[st.tile([128, 4 * HB], F32, tag=f"sig{bl}{p}",
                              name=f"sig{bl}{p}") for p in (0, 1)]
                     for bl in (0, 1)]
            th_v = [[st.tile([128, HB], F32, tag=f"th{bl}{p}",
                             name=f"th{bl}{p}") for p in (0, 1)]
                    for bl in (0, 1)]
            prod_v = [[st.tile([128, 2 * HB], F32, tag=f"prod{bl}{p}",
                               name=f"prod{bl}{p}") for p in (0, 1)]
                      for bl in (0, 1)]
            # ct: [c | tanh(g~)] per block, persistent fp32
            ct = [st.tile([128, 2 * HB], F32, tag=f"ct{bl}", name=f"ct{bl}")
                  for bl in (0, 1)]
            h_v = [st.tile([128, KC * BPC], BF16, tag=f"h{p}", name=f"h{p}")
                   for p in (0, 1)]
            hfin = st.tile([128, KC * BPC], F32)
            nc.vector.memset(ct[0][:], 0.0)
            nc.vector.memset(ct[1][:], 0.0)
            nc.vector.memset(h_v[0][:], 0.0)

            def chain_pre(ps, bl, s):
                """Sigmoid + c-update for block bl; h-halves 0/1.
                Block B's elementwise runs on gpsimd so it cannot delay
                block A's critical chain on the vector engine."""
                par = s % 2
                sa, prod = sig_v[bl][par], prod_v[bl][par]
                c = ct[bl]
                ve = nc.vector if bl == 0 else nc.gpsimd
                # [f | i | o | 2g~] -> sigmoid; tanh(g~) = 2*sig(2g~)-1
                nc.scalar.activation(sa[:], ps[:], AF.Sigmoid)
                ve.tensor_scalar(
                    c[:, HB:2 * HB], sa[:, 3 * HB:4 * HB], 2.0, -1.0,
                    OP.mult, OP.add,
                )
                # [f*c | i*g~] then c_new
                ve.tensor_mul(prod[:], sa[:, 0:2 * HB], c[:])
                ve.tensor_add(c[:, 0:HB], prod[:, 0:HB],
                              prod[:, HB:2 * HB])

            def chain_post(bl, s, last):
                par = s % 2
                sa, th = sig_v[bl][par], th_v[bl][par]
                h_new = h_v[(s + 1) % 2]
                lo, hi = bl * HB, (bl + 1) * HB
                nc.scalar.activation(th[:], ct[bl][:, 0:HB], AF.Tanh)
                if not last:
                    nc.vector.tensor_mul(h_new[:, lo:hi], sa[:, 2 * HB:3 * HB],
                                         th[:])
                else:
                    nc.vector.tensor_mul(hfin[:, lo:hi], sa[:, 2 * HB:3 * HB],
                                         th[:])
                    if bl == 1:
                        nc.sync.dma_start(hout_d[:], hfin[:])

            for c in range(nchunk):
                xp_sb = spool.tile([128, GC, chunk * BPC], BF16)
                for g in range(GC):
                    nc.sync.dma_start(
                        xp_sb[:, g, :],
                        xp_d[g, :, c * chunk * BPC:(c + 1) * chunk * BPC],
                    )
                p1ops = p1_micro_ops(c + 2) if c + 2 < nchunk else []
                p1pos = 0
                for s in range(chunk):
                    while p1pos < len(p1ops) and p1ops[p1pos][0] <= s:
                        p1ops[p1pos][1]()
                        p1pos += 1
                    t = c * chunk + s
                    h_cur = h_v[t % 2]
                    sl = slice(s * BPC, (s + 1) * BPC)
                    psA = gps.tile([128, 8 * BPC], F32)
                    psB = gps.tile([128, 8 * BPC], F32)
                    nc.tensor.matmul(psA[:], ident[:], xp_sb[:, 0:8, sl],
                                     start=True, stop=False,
                                     skip_group_check=True)
                    nc.tensor.matmul(psB[:], ident[:], xp_sb[:, 8:16, sl],
                                     start=True, stop=False,
                                     skip_group_check=True)
                    # k-half major: [A-k01, B-k01, A-k23, B-k23] so each
                    # psum block stops as early as its h inputs allow
                    for kh in range(2):
                        for bl, ps in ((0, psA), (1, psB)):
                            for k in (2 * kh, 2 * kh + 1):
                                for j in range(8):
                                    g = bl * 8 + j
                                    nc.tensor.matmul(
                                        ps[:, j * BPC:(j + 1) * BPC],
                                        whT[:, k * G + g * 128: k * G + (g + 1) * 128],
                                        h_cur[:, k * BPC:(k + 1) * BPC],
                                        start=False,
                                        stop=(k == KC - 1 and j == 7),
                                        skip_group_check=True,
                                    )
                    last = (c == nchunk - 1 and s == chunk - 1)
                    # emission order fixes per-engine queues:
                    # ACT: sigA, sigB, tanhA, tanhB
                    # DVE: t1A,mulA,addA, t1B,mulB,addB, mulhA, mulhB
                    chain_pre(psA, 0, t)
                    chain_pre(psB, 1, t)
                    chain_post(0, t, last)
                    chain_post(1, t, last)
                while p1pos < len(p1ops):
                    p1ops[p1pos][1]()
                    p1pos += 1

    nc.compile()
    return nc


def _prep_inputs(x_seq, W_hf, b_hf, W_xf, b_xf, W_hi, b_hi, W_xi, b_xi,
                 W_hg, b_hg, W_xg, b_xg, W_ho, b_ho, W_xo, b_xo, t_steps,
                 wdt):
    # gate order [f, i, o, g]; g~ weights/bias pre-scaled by 2
    Wx = np.concatenate([W_xf, W_xi, W_xo, 2.0 * W_xg], 0).astype(np.float32)
    Wh = np.concatenate([W_hf, W_hi, W_ho, 2.0 * W_hg], 0).astype(np.float32)
    bias = np.concatenate(
        [b_xf + b_hf, b_xi + b_hi, b_xo + b_ho, 2.0 * (b_xg + b_hg)], 0
    ).astype(np.float32)
    # permute 128-row chunks into [blockA | blockB] order
    Wx = Wx.reshape(GC, 128, D)[PERM].reshape(G, D)
    Wh = Wh.reshape(GC, 128, H)[PERM].reshape(G, H)
    bias = bias.reshape(GC, 128)[PERM]

    npw = npfp8 if wdt is FP8 else npbf16
    whT = np.ascontiguousarray(Wh.T.reshape(KC, 128, G)).astype(npw)
    wxT = np.ascontiguousarray(Wx.T.reshape(KC, 128, G)).astype(npbf16)
    biasr = np.ascontiguousarray(bias.T).astype(np.float32)
    ident = np.eye(128, dtype=np.float32).astype(npbf16)

    in_maps = []
    for i in range(NC):
        xc = np.asarray(x_seq[i * BPC:(i + 1) * BPC, :t_steps])  # [8, t, 512]
        xT = np.ascontiguousarray(
            xc.transpose(2, 1, 0).reshape(KC, 128, t_steps * BPC)
        ).astype(npbf16)
        in_maps.append({
            "xT": xT, "whT": whT, "wxT": wxT, "bias": biasr, "ident": ident,
        })
    return in_maps


def run_kernel(trace=False, t_steps=T, wdt=BF16, **inputs):
    key = (t_steps, wdt)
    if key not in _CACHE:
        _CACHE[key] = _build(t_steps, wdt)
    nc = _CACHE[key]
    in_maps = _prep_inputs(t_steps=t_steps, wdt=wdt, **inputs)
    res = bass_utils.run_bass_kernel_spmd(
        nc, in_maps, core_ids=list(range(NC)), trace=trace
    )
    outs = []
    for i in range(NC):
        r = np.asarray(res.results[i]["hout"])  # [128, 32]
        outs.append(r.reshape(128, KC, BPC).transpose(2, 1, 0).reshape(BPC, H))
    h = np.concatenate(outs, 0).astype(np.float32)
    return h, res


def kernel(**inputs):
    h, _ = run_kernel(trace=False, t_steps=T, **inputs)
    return h


# revision 20
# speedup vs baseline: 1.2253x; 1.2253x over previous
"""Trainium2 Bass kernel for the CustomLSTMCell problem.

B=64, T=1024, D=H=512.  Data-parallel over batch: 8 NeuronCores x 8 rows.

Per-core plan (matmul operands bf16 (or fp8 weights), state fp32):
  Host pre-transposes weights/x so no on-chip transposes are needed.
  Gate-chunk order is chosen so the per-step gate PSUM splits into two
  independently-completing half blocks:
    block A: h-dims 0..255   chunks [f0,f1,i0,i1,o0,o1,2g0,2g1]
    block B: h-dims 256..511 chunks [f2,f3,i2,i3,o2,o3,2g2,2g3]
  The g~ gate weights/biases are pre-scaled by 2 so ONE Sigmoid ACT per
  block covers all four gates: tanh(x) = 2*sigmoid(2x) - 1.

  Phase 1: x_proj[g,p,(t,b)] = Wx.T @ x + b  -> bf16 DRAM scratch.
  Phase 2: 1024 sequential steps.  Per step:
    - 2 identity matmuls deposit x_proj_t into psA/psB (start=True)
    - 32+32 (LDWEIGHTS+MATMUL) pairs accumulate Wh @ h_{t-1}; within a
      block the contraction chunks k go oldest-first so the next step's
      stream needs fresh h-chunks as late as possible; block A stops at
      half-stream so its activation chain overlaps block B's stream.
    - per block: Sigmoid ACT -> DVE (2s-1, muls, add) -> Tanh ACT -> DVE
      updates c (fp32) and h (bf16), software-pipelined across steps.
  x_proj is staged from DRAM in 128-step chunks, double buffered.
"""

import numpy as np
import ml_dtypes

import concourse.bass as bass
import concourse.bacc as bacc
import concourse.mybir as mybir
import concourse.tile as tile
import concourse.bass_utils as bass_utils

BF16 = mybir.dt.bfloat16
FP8 = mybir.dt.float8e4
F32 = mybir.dt.float32
AF = mybir.ActivationFunctionType
OP = mybir.AluOpType
npbf16 = ml_dtypes.bfloat16
npfp8 = ml_dtypes.float8_e4m3

B, T, D, H = 64, 1024, 512, 512
NC = 8
BPC = B // NC            # 8 batch rows per core
G = 4 * H                # 2048 gate rows
KC = D // 128            # 4 contraction chunks
GC = G // 128            # 16 gate chunks
CHUNK = 128              # timesteps per x_proj staging chunk
TG = 512                 # (t,b) columns per phase-1 psum tile

# gate-chunk permutation: orig chunk id = gate*4 + k, gates [f,i,o,g]
PERM = [0, 1, 4, 5, 8, 9, 12, 13, 2, 3, 6, 7, 10, 11, 14, 15]

_CACHE = {}


def _build(t_steps, wdt, chunk=CHUNK):
    nc = bacc.Bacc(
        "TRN2",
        target_bir_lowering=False,
        debug=False,
        enable_asserts=False,
        num_devices=NC,
    )
    nchunk = t_steps // chunk
    tg = min(TG, t_steps * BPC)
    ntg = (t_steps * BPC) // tg

    xT_d = nc.dram_tensor("xT", [KC, 128, t_steps * BPC], BF16, kind="ExternalInput")
    whT_d = nc.dram_tensor("whT", [KC, 128, G], wdt, kind="ExternalInput")
    wxT_d = nc.dram_tensor("wxT", [KC, 128, G], BF16, kind="ExternalInput")
    bias_d = nc.dram_tensor("bias", [128, GC], F32, kind="ExternalInput")
    ident_d = nc.dram_tensor("ident", [128, 128], BF16, kind="ExternalInput")
    hout_d = nc.dram_tensor("hout", [128, KC * BPC], F32, kind="ExternalOutput")

    with tile.TileContext(nc) as tc:
        with (
            tc.tile_pool(name="wpool", bufs=1) as wpool,
            tc.tile_pool(name="xpool", bufs=1) as xpool,
            tc.tile_pool(name="xpd", bufs=1, space="DRAM") as dpool,
            tc.tile_pool(name="p1ps", bufs=2, space="PSUM") as p1ps,
            tc.tile_pool(name="p1out", bufs=4) as p1out,
            tc.tile_pool(name="stage", bufs=2) as spool,
            tc.tile_pool(name="gps", bufs=2, space="PSUM") as gps,
            tc.tile_pool(name="state", bufs=1) as st,
        ):
            # ---- resident tensors ----
            whT = wpool.tile([128, KC * G], wdt)
            wxT = wpool.tile([128, KC * G], BF16)
            biasr = wpool.tile([128, GC], F32)
            ident = wpool.tile([128, 128], BF16)
            for k in range(KC):
                nc.sync.dma_start(whT[:, k * G:(k + 1) * G], whT_d[k])
                nc.sync.dma_start(wxT[:, k * G:(k + 1) * G], wxT_d[k])
            nc.sync.dma_start(biasr[:], bias_d[:])
            nc.sync.dma_start(ident[:], ident_d[:])

            xT = xpool.tile([128, KC * t_steps * BPC], BF16)
            W = t_steps * BPC
            for k in range(KC):
                nc.sync.dma_start(xT[:, k * W:(k + 1) * W], xT_d[k])

            xp_d = dpool.tile([GC, 128, t_steps * BPC], BF16)

            # ---- phase 1: x projection (micro-op emitters) ----
            # A "group" is one (tgi, g) output tile: 4 accumulating MMs,
            # 4 bias sub-adds of 128 cols, 1 DMA out.  The first two
            # chunks' groups run upfront; later chunks' groups are
            # interleaved into the recurrence steps (PE/DVE idle time).
            p1_state = {}

            def p1_mm(tgi, g, k):
                if k == 0:
                    p1_state[(tgi, g)] = (
                        p1ps.tile([128, tg], F32, name="p1ps_t"),
                        p1out.tile([128, tg], BF16, name="p1ev_t"),
                    )
                ps, _ = p1_state[(tgi, g)]
                nc.tensor.matmul(
                    ps[:],
                    wxT[:, k * G + g * 128: k * G + (g + 1) * 128],
                    xT[:, k * W + tgi * tg: k * W + (tgi + 1) * tg],
                    start=(k == 0),
                    stop=(k == KC - 1),
                )

            def p1_bias(tgi, g, j):
                ps, ev = p1_state[(tgi, g)]
                q = tg // 4
                nc.vector.tensor_scalar_add(
                    ev[:, j * q:(j + 1) * q], ps[:, j * q:(j + 1) * q],
                    biasr[:, g:g + 1])

            def p1_dma(tgi, g):
                _, ev = p1_state.pop((tgi, g))
                nc.sync.dma_start(xp_d[g, :, tgi * tg:(tgi + 1) * tg], ev[:])

            def p1_group(tgi, g):
                for k in range(KC):
                    p1_mm(tgi, g, k)
                for j in range(4):
                    p1_bias(tgi, g, j)
                p1_dma(tgi, g)

            tgi_per_chunk = (chunk * BPC) // tg
            up_chunks = min(2, nchunk)
            for tgi in range(up_chunks * tgi_per_chunk):
                for g in range(GC):
                    p1_group(tgi, g)

            def p1_micro_ops(n):
                """Micro-op list for source chunk n, consumed one per step."""
                ops = []
                base = n * tgi_per_chunk
                ngroups = tgi_per_chunk * GC
                for gi in range(ngroups):
                    tgi, g = base + gi // GC, gi % GC
                    for k in range(KC):
                        ops.append((4 * gi + k, lambda t=tgi, gg=g, kk=k:
                                    p1_mm(t, gg, kk)))
                    for j in range(4):
                        ops.append((4 * gi + 5 + j, lambda t=tgi, gg=g, jj=j:
                                    p1_bias(t, gg, jj)))
                    ops.append((4 * gi + 9, lambda t=tgi, gg=g: p1_dma(t, gg)))
                ops.sort(key=lambda x: x[0])
                return ops

            # ---- phase 2: recurrence ----
            HB = 2 * BPC  # 16: free width of one h half (2 k-chunks x 8 b)
            # per-block state, double-buffered by step parity where written
            # by one engine and read by another across steps
            sig_v = [[st.tile([128, 4 * HB], F32, tag=f"sig{bl}{p}",
                              name=f"sig{bl}{p}") for p in (0, 1)]
                     for bl in (0, 1)]
            th_v = [[st.tile([128, HB], F32, tag=f"th{bl}{p}",
                             name=f"th{bl}{p}") for p in (0, 1)]
                    for bl in (0, 1)]
            prod_v = [[st.tile([128, 2 * HB], F32, tag=f"prod{bl}{p}",
                               name=f"prod{bl}{p}") for p in (0, 1)]
                      for bl in (0, 1)]
            # ct: [c | tanh(g~)] per block, persistent fp32
            ct = [st.tile([128, 2 * HB], F32, tag=f"ct{bl}", name=f"ct{bl}")
                  for bl in (0, 1)]
            h_v = [st.tile([128, KC * BPC], BF16, tag=f"h{p}", name=f"h{p}")
                   for p in (0, 1)]
            hfin = st.tile([128, KC * BPC], F32)
            nc.vector.memset(ct[0][:], 0.0)
            nc.vector.memset(ct[1][:], 0.0)
            nc.vector.memset(h_v[0][:], 0.0)

            def chain_pre(ps, bl, s):
                """Sigmoid + c-update for block bl; h-halves 0/1.
                Block B's elementwise runs on gpsimd so it cannot delay
                block A's critical chain on the vector engine."""
                par = s % 2
                sa, prod = sig_v[bl][par], prod_v[bl][par]
                c = ct[bl]
                ve = nc.vector if bl == 0 else nc.gpsimd
                # [f | i | o | 2g~] -> sigmoid; tanh(g~) = 2*sig(2g~)-1
                nc.scalar.activation(sa[:], ps[:], AF.Sigmoid)
                ve.tensor_scalar(
                    c[:, HB:2 * HB], sa[:, 3 * HB:4 * HB], 2.0, -1.0,
                    OP.mult, OP.add,
                )
                # [f*c | i*g~] then c_new
                ve.tensor_mul(prod[:], sa[:, 0:2 * HB], c[:])
                ve.tensor_add(c[:, 0:HB], prod[:, 0:HB],
                              prod[:, HB:2 * HB])

            def chain_post(bl, s, last):
                par = s % 2
                sa, th = sig_v[bl][par], th_v[bl][par]
                h_new = h_v[(s + 1) % 2]
                lo, hi = bl * HB, (bl + 1) * HB
                nc.scalar.activation(th[:], ct[bl][:, 0:HB], AF.Tanh)
                if not last:
                    nc.vector.tensor_mul(h_new[:, lo:hi], sa[:, 2 * HB:3 * HB],
                                         th[:])
                else:
                    nc.vector.tensor_mul(hfin[:, lo:hi], sa[:, 2 * HB:3 * HB],
                                         th[:])
                    if bl == 1:
                        nc.sync.dma_start(hout_d[:], hfin[:])

            for c in range(nchunk):
                xp_sb = spool.tile([128, GC, chunk * BPC], BF16)
                for g in range(GC):
                    nc.sync.dma_start(
                        xp_sb[:, g, :],
                        xp_d[g, :, c * chunk * BPC:(c + 1) * chunk * BPC],
                    )
                p1ops = p1_micro_ops(c + 2) if c + 2 < nchunk else []
                p1pos = 0
                for s in range(chunk):
                    while p1pos < len(p1ops) and p1ops[p1pos][0] <= s:
                        p1ops[p1pos][1]()
                        p1pos += 1
                    t = c * chunk + s
                    h_cur = h_v[t % 2]
                    sl = slice(s * BPC, (s + 1) * BPC)
                    psA = gps.tile([128, 8 * BPC], F32)
                    psB = gps.tile([128, 8 * BPC], F32)
                    nc.tensor.matmul(psA[:], ident[:], xp_sb[:, 0:8, sl],
                                     start=True, stop=False,
                                     skip_group_check=True)
                    nc.tensor.matmul(psB[:], ident[:], xp_sb[:, 8:16, sl],
                                     start=True, stop=False,
                                     skip_group_check=True)
                    # k-half major: [A-k01, B-k01, A-k23, B-k23] so each
                    # psum block stops as early as its h inputs allow
                    for kh in range(2):
                        for bl, ps in ((0, psA), (1, psB)):
                            for k in (2 * kh, 2 * kh + 1):
                                for j in range(8):
                                    g = bl * 8 + j
                                    nc.tensor.matmul(
                                        ps[:, j * BPC:(j + 1) * BPC],
                                        whT[:, k * G + g * 128: k * G + (g + 1) * 128],
                                        h_cur[:, k * BPC:(k + 1) * BPC],
                                        start=False,
                                        stop=(k == KC - 1 and j == 7),
                                        skip_group_check=True,
                                    )
                    last = (c == nchunk - 1 and s == chunk - 1)
                    # emission order fixes per-engine queues:
                    # ACT: sigA, sigB, tanhA, tanhB
                    # DVE: t1A,mulA,addA, t1B,mulB,addB, mulhA, mulhB
                    chain_pre(psA, 0, t)
                    chain_pre(psB, 1, t)
                    chain_post(0, t, last)
                    chain_post(1, t, last)
                while p1pos < len(p1ops):
                    p1ops[p1pos][1]()
                    p1pos += 1

    nc.compile()
    return nc


def _prep_inputs(x_seq, W_hf, b_hf, W_xf, b_xf, W_hi, b_hi, W_xi, b_xi,
                 W_hg, b_hg, W_xg, b_xg, W_ho, b_ho, W_xo, b_xo, t_steps,
                 wdt):
    # gate order [f, i, o, g]; g~ weights/bias pre-scaled by 2
    Wx = np.concatenate([W_xf, W_xi, W_xo, 2.0 * W_xg], 0).astype(np.float32)
    Wh = np.concatenate([W_hf, W_hi, W_ho, 2.0 * W_hg], 0).astype(np.float32)
    bias = np.concatenate(
        [b_xf + b_hf, b_xi + b_hi, b_xo + b_ho, 2.0 * (b_xg + b_hg)], 0
    ).astype(np.float32)
    # permute 128-row chunks into [blockA | blockB] order
    Wx = Wx.reshape(GC, 128, D)[PERM].reshape(G, D)
    Wh = Wh.reshape(GC, 128, H)[PERM].reshape(G, H)
    bias = bias.reshape(GC, 128)[PERM]

    npw = npfp8 if wdt is FP8 else npbf16
    whT = np.ascontiguousarray(Wh.T.reshape(KC, 128, G)).astype(npw)
    wxT = np.ascontiguousarray(Wx.T.reshape(KC, 128, G)).astype(npbf16)
    biasr = np.ascontiguousarray(bias.T).astype(np.float32)
    ident = np.eye(128, dtype=np.float32).astype(npbf16)

    in_maps = []
    for i in range(NC):
        xc = np.asarray(x_seq[i * BPC:(i + 1) * BPC, :t_steps])  # [8, t, 512]
        xT = np.ascontiguousarray(
            xc.transpose(2, 1, 0).reshape(KC, 128, t_steps * BPC)
        ).astype(npbf16)
        in_maps.append({
            "xT": xT, "whT": whT, "wxT": wxT, "bias": biasr, "ident": ident,
        })
    return in_maps


def run_kernel(trace=False, t_steps=T, wdt=BF16, **inputs):
    key = (t_steps, wdt)
    if key not in _CACHE:
        _CACHE[key] = _build(t_steps, wdt)
    nc = _CACHE[key]
    in_maps = _prep_inputs(t_steps=t_steps, wdt=wdt, **inputs)
    res = bass_utils.run_bass_kernel_spmd(
        nc, in_maps, core_ids=list(range(NC)), trace=trace
    )
    outs = []
    for i in range(NC):
        r = np.asarray(res.results[i]["hout"])  # [128, 32]
        outs.append(r.reshape(128, KC, BPC).transpose(2, 1, 0).reshape(BPC, H))
    h = np.concatenate(outs, 0).astype(np.float32)
    return h, res


def kernel(**inputs):
    h, _ = run_kernel(trace=False, t_steps=T, **inputs)
    return h
